# revision 2
# baseline (speedup 1.0000x reference)
"""BinaryTreeLSTM Trainium2 kernel — data-parallel over the batch (tree) axis.

Layout strategy: activations in [feature, row] layout on-chip; TensorE
matmul PSUM output [out_feat, row] is exactly the next level's moving
operand layout, so no on-device transposes.  The host transposes embs
once and packs/casts weights.

v2 structure (vs v1 baseline):
 - Leaf biases are folded into the K=45 tail matmul (host appends a
   ones-row to embs and a bias-row to the tail weights), so the leaf
   needs no per-element bias add and the two [128,512] PSUM banks of
   each gate can be activated with ONE [128,1024] instruction.
 - PSUM is organized as four 2-bank [128,1024] pair tiles.
 - Per level, both feature halves live in ONE tile ([128, 2, rows]),
   so each DVE c-chain op covers both halves in one instruction
   (amortizes the ~150-cycle per-op bubble).
 - Gates are stored bf16 (enables 16-bit DVE/ACT fast paths; rel-err
   budget checked by simulation: ~1.4e-2 vs the 2e-2 gate).
 - PE warm-up: gpsimd memset (earliest engine ready) + 10 dummy
   matmuls so HAM un-throttles before real work lands.
 - Tail levels 5..8 run as single wide compose calls.

dtypes: matmul operands bf16 (fp32 PSUM accumulate); cell state c and
the DVE chain stay fp32 end-to-end; gates/h stored bf16.
"""

import sys

if "/opt/trn_rl_repo" not in sys.path:
    sys.path.insert(0, "/opt/trn_rl_repo")

from contextlib import ExitStack

import ml_dtypes
import numpy as np

N_CORES = 8
B, L, IN, M = 512, 256, 300, 256
BC = B // N_CORES            # trees per core
LEAF_ROWS = BC * L           # 16384 leaf rows per core
T_CHUNK = 8                  # trees per chunk
N_CHUNKS = BC // T_CHUNK
CHUNK_LEAF = T_CHUNK * L     # 2048 leaf rows per chunk
N_DUMMY = 10                 # HAM warm-up matmuls

_CACHE = {}
LAST_RESULTS = None


def _build():
    import concourse.bass as bass  # noqa: F401
    import concourse.tile as tile
    from concourse import bacc, mybir

    F32 = mybir.dt.float32
    BF16 = mybir.dt.bfloat16
    SIG = mybir.ActivationFunctionType.Sigmoid
    TANH = mybir.ActivationFunctionType.Tanh

    nc = bacc.Bacc("TRN2", target_bir_lowering=False, debug=False,
                   num_devices=N_CORES)
    emb_d = nc.dram_tensor("embs_t", [IN + 1, LEAF_ROWS], BF16,
                           kind="ExternalInput").ap()
    wl_d = nc.dram_tensor("w_leaf", [128, 1024], BF16,
                          kind="ExternalInput").ap()
    wl2_d = nc.dram_tensor("w_leaf2", [128, 2 * M], BF16,
                           kind="ExternalInput").ap()
    wc_d = nc.dram_tensor("w_comp", [2 * M, 4 * M], BF16,
                          kind="ExternalInput").ap()
    b_d = nc.dram_tensor("biases", [128, 8], F32, kind="ExternalInput").ap()
    outc_d = nc.dram_tensor("out_c", [M, BC], F32, kind="ExternalOutput").ap()
    outh_d = nc.dram_tensor("out_h", [M, BC], F32, kind="ExternalOutput").ap()

    with tile.TileContext(nc) as tc, ExitStack() as ctx:
        wpool = ctx.enter_context(tc.tile_pool(name="w", bufs=1))
        xpool = ctx.enter_context(tc.tile_pool(name="x", bufs=5))
        lvl = ctx.enter_context(tc.tile_pool(name="lvl", bufs=1))
        gp = ctx.enter_context(tc.tile_pool(name="g", bufs=2))
        pp = ctx.enter_context(tc.tile_pool(name="ps", bufs=1, space="PSUM"))

        # --- PE warm-up: memset on gpsimd (earliest engine out of its
        # preamble) so dummy matmuls can start ~6us in, covering the
        # initial DMA wait and warming the HAM clock gate ---
        dummy = wpool.tile([128, 512], BF16, name="dummy", tag="dummy")
        nc.gpsimd.memset(dummy[:], 0.0)

        # --- weights / biases (resident) on the ACT HWDGE queue; the SP
        # queue leads with the first leaf's embs tiles ---
        wl = wpool.tile([128, 1024], BF16, name="wl", tag="wl")
        nc.scalar.dma_start(wl[:], wl_d[:, :])
        wl2 = wpool.tile([128, 2 * M], BF16, name="wl2", tag="wl2")
        nc.scalar.dma_start(wl2[:], wl2_d[:, :])
        bias = wpool.tile([128, 8], F32, name="bias", tag="bias")
        nc.scalar.dma_start(bias[:], b_d[:, :])
        wc = [wpool.tile([128, 4 * M], BF16, name=f"wc{k}", tag=f"wc{k}")
              for k in range(4)]

        # PSUM: four [128, 1024] 2-bank pair tiles
        pairs = [pp.tile([128, 1024], F32, name=f"pp{i}", tag=f"pp{i}")
                 for i in range(4)]

        def ps(mt, n=512):
            return pairs[mt // 2][:, (mt % 2) * 512:(mt % 2) * 512 + n]

        # dummies on pairs 2,3 (banks 4-7); first leaf sub uses pairs 0,1
        for i in range(N_DUMMY):
            nc.tensor.matmul(ps(4 + (i % 4)), dummy[:, 0:128],
                             dummy[:], start=True, stop=True)

        # --- parked L3 state (all trees) ---
        pk_h = lvl.tile([128, 2, BC * 32], BF16, name="pkh", tag="pkh")
        pk_c = lvl.tile([128, 2, BC * 32], F32, name="pkc", tag="pkc")

        # gate m-tile order: wc columns are [i0,i1,lf0,lf1,rf0,rf1,u0,u1];
        # i and u first so the DVE t1 = i*u can start earliest.
        MT_ORDER = (0, 1, 6, 7, 2, 3, 4, 5)
        GATE_OF_MT = (0, 0, 1, 1, 2, 2, 3, 3)   # i, lf, rf, u

        def _gates_and_chain(ce, houts, couts, n, defer_h):
            """Gate activations + c-chain for one <=512-node sub-chunk.
            ce: [128, 2, 2, n] view of prev c (dim2 = even/odd);
            houts/couts: [128, 2, n] views."""
            g = [None] * 4   # i, lf, rf, u tiles [128, 2, 512] bf16
            for mt in MT_ORDER:
                gi, half = GATE_OF_MT[mt], mt % 2
                if g[gi] is None:
                    g[gi] = gp.tile([128, 2, 512], BF16, name=f"g{gi}",
                                    tag=f"g{gi}", bufs=2)
                fn = TANH if gi == 3 else SIG
                nc.scalar.activation(g[gi][:, half, :n], ps(mt, n), fn,
                                     bias=bias[:, mt:mt + 1])
            gv = [t[:, :, :n] for t in g]
            t1 = gp.tile([128, 2, 512], BF16, name="t1", tag="t1", bufs=2)
            nc.vector.tensor_mul(t1[:, :, :n], gv[0], gv[3])
            t2 = gp.tile([128, 2, 512], F32, name="t2", tag="t2", bufs=1)
            nc.vector.tensor_mul(t2[:, :, :n], gv[1], ce[:, :, 0, :])
            t3 = gp.tile([128, 2, 512], F32, name="t3", tag="t3", bufs=1)
            nc.vector.tensor_mul(t3[:, :, :n], gv[2], ce[:, :, 1, :])
            s12 = gp.tile([128, 2, 512], F32, name="s12", tag="s12", bufs=1)
            nc.vector.tensor_add(s12[:, :, :n], t1[:, :, :n], t2[:, :, :n])
            nc.vector.tensor_add(couts, s12[:, :, :n], t3[:, :, :n])
            if not defer_h:
                nc.scalar.activation(houts, couts, TANH)

        def compose(hp, cp, houts, couts, n, defer_h=False):
            """One compose sub-chunk (n <= 512 output nodes).
            hp/cp: [128, 2, 2n] views of the previous level."""
            hv = hp.rearrange("p f (n two) -> p f two n", two=2)
            ce = cp.rearrange("p f (n two) -> p f two n", two=2)
            rhs = [hv[:, 0, 0, :], hv[:, 1, 0, :],
                   hv[:, 0, 1, :], hv[:, 1, 1, :]]
            for mt in MT_ORDER:
                for k in range(4):
                    nc.tensor.matmul(ps(mt, n),
                                     wc[k][:, mt * 128:(mt + 1) * 128],
                                     rhs[k], start=(k == 0), stop=(k == 3))
            _gates_and_chain(ce, houts, couts, n, defer_h)

        st = {}  # (level, ch) -> (h_tile, c_tile)

        def emit_leaf(ch):
            h_lf = lvl.tile([128, 2, CHUNK_LEAF], BF16, name="hlf", tag="hlf")
            c_lf = lvl.tile([128, 2, CHUNK_LEAF], F32, name="clf", tag="clf")
            for s in range(CHUNK_LEAF // 512):
                col0 = ch * CHUNK_LEAF + s * 512
                xk01 = xpool.tile([128, 1024], BF16, name="xk01", tag="xk01")
                nc.sync.dma_start(
                    xk01.rearrange("p (k n) -> p k n", k=2),
                    emb_d[0:256, col0:col0 + 512].rearrange(
                        "(k p) n -> p k n", p=128))
                xk2 = xpool.tile([128, 512], BF16, name="xk2", tag="xk2")
                nc.sync.dma_start(xk2[0:45, :],
                                  emb_d[256:301, col0:col0 + 512])
                nc.sync.dma_start(xk2[64:109, :],
                                  emb_d[256:301, col0:col0 + 512])
                po = 2 * (s % 2)      # pairs 0,1 or 2,3
                for mt in range(4):
                    for k in range(2):
                        nc.tensor.matmul(
                            ps(2 * po + mt),
                            wl[:, k * 512 + mt * 128:k * 512 + (mt + 1) * 128],
                            xk01[:, k * 512:(k + 1) * 512],
                            start=(k == 0), stop=False)
                # K=45 tail (44 emb rows + bias row); two m-tiles run
                # concurrently in disjoint PE row groups
                for mt in range(0, 4, 2):
                    nc.tensor.matmul(
                        ps(2 * po + mt), wl2[0:45, mt * 128:(mt + 1) * 128],
                        xk2[0:45, :], start=False, stop=True,
                        tile_position=(0, 0))
                    nc.tensor.matmul(
                        ps(2 * po + mt + 1),
                        wl2[64:109, (mt + 1) * 128:(mt + 2) * 128],
                        xk2[64:109, :], start=False, stop=True,
                        tile_position=(64, 0))
                dst = slice(s * 512, (s + 1) * 512)
                tcell = gp.tile([128, 1024], BF16, name="lf_tc",
                                tag="lf_tc", bufs=2)
                nc.scalar.activation(tcell[:], pairs[po][:], TANH)
                to = gp.tile([128, 1024], BF16, name="lf_to",
                             tag="lf_to", bufs=2)
                nc.scalar.activation(to[:], pairs[po + 1][:], SIG)
                nc.vector.tensor_copy(
                    c_lf[:, :, dst],
                    pairs[po].rearrange("p (f n) -> p f n", f=2))
                nc.vector.tensor_mul(
                    h_lf[:, :, dst],
                    to.rearrange("p (f n) -> p f n", f=2),
                    tcell.rearrange("p (f n) -> p f n", f=2))
            st[(0, ch)] = (h_lf, c_lf)

        def emit_level(li, ch):
            """Compose level li (1..3) of chunk ch."""
            prev_h, prev_c = st.pop((li - 1, ch))
            rows = CHUNK_LEAF >> li
            if li < 3:
                nh = lvl.tile([128, 2, rows], BF16, name=f"h{li}",
                              tag=f"h{li}", bufs=2)
                ncr = lvl.tile([128, 2, rows], F32, name=f"c{li}",
                               tag=f"c{li}", bufs=2)
                off = 0
            else:
                nh, ncr, off = pk_h, pk_c, ch * (CHUNK_LEAF // 8)
            for j in range(max(1, rows // 512)):
                n = min(512, rows - j * 512)
                compose(prev_h[:, :, j * 1024:j * 1024 + 2 * n],
                        prev_c[:, :, j * 1024:j * 1024 + 2 * n],
                        nh[:, :, off + j * 512:off + j * 512 + n],
                        ncr[:, :, off + j * 512:off + j * 512 + n],
                        n, defer_h=True)
            nc.scalar.activation(nh[:, :, off:off + rows],
                                 ncr[:, :, off:off + rows], TANH)
            if li < 3:
                st[(li, ch)] = (nh, ncr)

        # software-pipelined chunk schedule
        for ch in range(N_CHUNKS):
            emit_leaf(ch)
            if ch == 0:
                for k in range(4):
                    nc.scalar.dma_start(wc[k][:],
                                        wc_d[k * 128:(k + 1) * 128, :])
            if ch >= 1:
                emit_level(2, ch - 1)
            emit_level(1, ch)
            if ch >= 2:
                emit_level(3, ch - 2)
        emit_level(2, N_CHUNKS - 1)
        emit_level(3, N_CHUNKS - 2)

        # ---- tail levels 4..8 across all trees ----
        def tail_tiles(li, rows):
            nh = lvl.tile([128, 2, rows], BF16, name=f"h{li}", tag=f"h{li}")
            ncr = lvl.tile([128, 2, rows], F32, name=f"c{li}", tag=f"c{li}")
            return nh, ncr

        def tail_sub(prev, cur, j, n):
            prev_h, prev_c = prev
            nh, ncr = cur
            houts = nh[:, :, j * n:j * n + n]
            couts = ncr[:, :, j * n:j * n + n]
            compose(prev_h[:, :, j * 2 * n:j * 2 * n + 2 * n],
                    prev_c[:, :, j * 2 * n:j * 2 * n + 2 * n],
                    houts, couts, n, defer_h=True)
            nc.scalar.activation(houts, couts, TANH)

        l4 = tail_tiles(4, BC * 16)
        l5 = tail_tiles(5, BC * 8)
        l6 = tail_tiles(6, BC * 4)
        l7 = tail_tiles(7, BC * 2)
        tail_sub((pk_h, pk_c), l4, 0, 512)
        emit_level(3, N_CHUNKS - 1)
        tail_sub((pk_h, pk_c), l4, 2, 256)
        tail_sub((pk_h, pk_c), l4, 3, 256)
        tail_sub(l4, l5, 0, 512)
        tail_sub(l5, l6, 0, 256)
        tail_sub(l6, l7, 0, 128)
        stage_c = lvl.tile([128, 2, BC], F32, name="stc", tag="stc")
        stage_h = lvl.tile([128, 2, BC], F32, name="sth", tag="sth")
        compose(l7[0][:, :, 0:128], l7[1][:, :, 0:128],
                stage_h[:, :, 0:BC], stage_c[:, :, 0:BC], BC)
        for p in range(2):
            nc.sync.dma_start(outc_d[p * 128:(p + 1) * 128, :],
                              stage_c[:, p, :])
            nc.sync.dma_start(outh_d[p * 128:(p + 1) * 128, :],
                              stage_h[:, p, :])

    nc.compile()
    return nc


def _prep_inputs(embs, cx_w, cx_b, ox_w, ox_b, lh_w, lh_b, rh_w, rh_b):
    bf16 = ml_dtypes.bfloat16
    w_leaf = np.ascontiguousarray(
        np.concatenate([cx_w, ox_w], axis=0).T).astype(np.float32)  # [300,512]
    leaf_b = np.concatenate([cx_b, ox_b]).astype(np.float32)        # [512]
    # main k-tiles side by side: [128, 1024] = [k0 | k1]
    wl = np.concatenate([w_leaf[0:128], w_leaf[128:256]],
                        axis=1).astype(bf16)
    # K=45 tail (rows 256:300 + bias row), duplicated at partitions 64..108
    wl2 = np.zeros((128, 2 * M), np.float32)
    wl2[0:44] = w_leaf[256:300]
    wl2[44] = leaf_b
    wl2[64:108] = w_leaf[256:300]
    wl2[108] = leaf_b
    wl2 = wl2.astype(bf16)
    w_comp = np.ascontiguousarray(np.concatenate(
        [lh_w.reshape(4 * M, M).T, rh_w.reshape(4 * M, M).T],
        axis=0)).astype(bf16)                                       # [512,1024]
    b_comp = (lh_b + rh_b).reshape(-1).astype(np.float32)           # [1024]
    biases = np.ascontiguousarray(b_comp.reshape(8, 128).T)         # [128, 8]
    embs_sh = np.asarray(embs, np.float32).reshape(N_CORES, LEAF_ROWS, IN)
    in_maps = []
    for c in range(N_CORES):
        xt = np.empty((IN + 1, LEAF_ROWS), np.float32)
        xt[0:IN] = embs_sh[c].T
        xt[IN] = 1.0
        in_maps.append({"embs_t": np.ascontiguousarray(xt.astype(bf16)),
                        "w_leaf": wl, "w_leaf2": wl2,
                        "w_comp": w_comp, "biases": biases})
    return in_maps


def kernel(embs, cx_w, cx_b, ox_w, ox_b, lh_w, lh_b, rh_w, rh_b):
    global LAST_RESULTS
    from concourse.bass_utils import run_bass_kernel_spmd

    if "nc" not in _CACHE:
        _CACHE["nc"] = _build()
    nc = _CACHE["nc"]
    in_maps = _prep_inputs(embs, cx_w, cx_b, ox_w, ox_b,
                           lh_w, lh_b, rh_w, rh_b)
    res = run_bass_kernel_spmd(nc, in_maps, core_ids=list(range(N_CORES)))
    LAST_RESULTS = res
    c_out = np.empty((B, M), np.float32)
    h_out = np.empty((B, M), np.float32)
    for c in range(N_CORES):
        c_out[c * BC:(c + 1) * BC] = res.results[c]["out_c"].T
        h_out[c * BC:(c + 1) * BC] = res.results[c]["out_h"].T
    return c_out, h_out


# revision 8
# speedup vs baseline: 1.0276x; 1.0276x over previous
"""BinaryTreeLSTM Trainium2 kernel — data-parallel over the batch (tree) axis.

Layout strategy: activations in [feature, row] layout on-chip; TensorE
matmul PSUM output [out_feat, row] is exactly the next level's moving
operand layout, so no on-device transposes.  The host transposes embs
once and packs/casts weights.

v2 structure (vs v1 baseline):
 - Leaf biases are folded into the K=45 tail matmul (host appends a
   ones-row to embs and a bias-row to the tail weights), so the leaf
   needs no per-element bias add and the two [128,512] PSUM banks of
   each gate can be activated with ONE [128,1024] instruction.
 - PSUM is organized as four 2-bank [128,1024] pair tiles.
 - Per level, both feature halves live in ONE tile ([128, 2, rows]),
   so each DVE c-chain op covers both halves in one instruction
   (amortizes the ~150-cycle per-op bubble).
 - Gates are stored bf16 (enables 16-bit DVE/ACT fast paths; rel-err
   budget checked by simulation: ~1.4e-2 vs the 2e-2 gate).
 - PE warm-up: gpsimd memset (earliest engine ready) + 10 dummy
   matmuls so HAM un-throttles before real work lands.
 - Tail levels 5..8 run as single wide compose calls.

dtypes: matmul operands bf16 (fp32 PSUM accumulate); cell state c and
the DVE chain stay fp32 end-to-end; gates/h stored bf16.
"""

import sys

if "/opt/trn_rl_repo" not in sys.path:
    sys.path.insert(0, "/opt/trn_rl_repo")

from contextlib import ExitStack

import ml_dtypes
import numpy as np

N_CORES = 8
B, L, IN, M = 512, 256, 300, 256
BC = B // N_CORES            # trees per core
LEAF_ROWS = BC * L           # 16384 leaf rows per core
T_CHUNK = 8                  # trees per chunk
N_CHUNKS = BC // T_CHUNK
CHUNK_LEAF = T_CHUNK * L     # 2048 leaf rows per chunk
N_DUMMY = 6                  # HAM warm-up matmuls

_CACHE = {}
LAST_RESULTS = None


def _build():
    import concourse.bass as bass  # noqa: F401
    import concourse.tile as tile
    from concourse import bacc, mybir

    F32 = mybir.dt.float32
    BF16 = mybir.dt.bfloat16
    SIG = mybir.ActivationFunctionType.Sigmoid
    TANH = mybir.ActivationFunctionType.Tanh

    nc = bacc.Bacc("TRN2", target_bir_lowering=False, debug=False,
                   num_devices=N_CORES)
    emb_d = nc.dram_tensor("embs_t", [IN + 1, LEAF_ROWS], BF16,
                           kind="ExternalInput").ap()
    wl_d = nc.dram_tensor("w_leaf", [128, 1024], BF16,
                          kind="ExternalInput").ap()
    wl2_d = nc.dram_tensor("w_leaf2", [128, 2 * M], BF16,
                           kind="ExternalInput").ap()
    wc_d = nc.dram_tensor("w_comp", [2 * M, 4 * M], BF16,
                          kind="ExternalInput").ap()
    b_d = nc.dram_tensor("biases", [128, 8], F32, kind="ExternalInput").ap()
    outc_d = nc.dram_tensor("out_c", [M, BC], F32, kind="ExternalOutput").ap()
    outh_d = nc.dram_tensor("out_h", [M, BC], F32, kind="ExternalOutput").ap()

    with tile.TileContext(nc) as tc, ExitStack() as ctx:
        wpool = ctx.enter_context(tc.tile_pool(name="w", bufs=1))
        xpool = ctx.enter_context(tc.tile_pool(name="x", bufs=5))
        lvl = ctx.enter_context(tc.tile_pool(name="lvl", bufs=1))
        gp = ctx.enter_context(tc.tile_pool(name="g", bufs=2))
        pp = ctx.enter_context(tc.tile_pool(name="ps", bufs=1, space="PSUM"))

        # --- PE warm-up: memset on gpsimd (earliest engine out of its
        # preamble) so dummy matmuls can start ~6us in, covering the
        # initial DMA wait and warming the HAM clock gate ---
        dummy = wpool.tile([128, 512], BF16, name="dummy", tag="dummy")
        nc.gpsimd.memset(dummy[:], 0.0)

        # --- weights / biases (resident) on the ACT HWDGE queue; the SP
        # queue leads with the first leaf's embs tiles ---
        wl = wpool.tile([128, 1024], BF16, name="wl", tag="wl")
        nc.scalar.dma_start(wl[:], wl_d[:, :])
        wl2 = wpool.tile([128, 2 * M], BF16, name="wl2", tag="wl2")
        nc.scalar.dma_start(wl2[:], wl2_d[:, :])
        bias = wpool.tile([128, 8], F32, name="bias", tag="bias")
        nc.scalar.dma_start(bias[:], b_d[:, :])
        wc = [wpool.tile([128, 4 * M], BF16, name=f"wc{k}", tag=f"wc{k}")
              for k in range(4)]

        # PSUM: four [128, 1024] 2-bank pair tiles
        pairs = [pp.tile([128, 1024], F32, name=f"pp{i}", tag=f"pp{i}")
                 for i in range(4)]

        def ps(mt, n=512):
            return pairs[mt // 2][:, (mt % 2) * 512:(mt % 2) * 512 + n]

        # dummies on pairs 2,3 (banks 4-7); first leaf sub uses pairs 0,1
        for i in range(N_DUMMY):
            nc.tensor.matmul(ps(4 + (i % 4)), dummy[:, 0:128],
                             dummy[:], start=True, stop=True)

        # --- parked L3 state (all trees) ---
        pk_h = lvl.tile([128, 2, BC * 32], BF16, name="pkh", tag="pkh")
        pk_c = lvl.tile([128, 2, BC * 32], F32, name="pkc", tag="pkc")

        # gate m-tile order: wc columns are [i0,i1,lf0,lf1,rf0,rf1,u0,u1];
        # i and u first so the DVE t1 = i*u can start earliest.
        MT_ORDER = (0, 1, 6, 7, 2, 3, 4, 5)
        GATE_OF_MT = (0, 0, 1, 1, 2, 2, 3, 3)   # i, lf, rf, u

        def _gates_and_chain(ce, houts, couts, n, defer_h):
            """Gate activations + c-chain for one <=512-node sub-chunk.
            ce: [128, 2, 2, n] view of prev c (dim2 = even/odd);
            houts/couts: [128, 2, n] views."""
            g = [None] * 4   # i, lf, rf, u tiles [128, 2, 512] fp32
            for mt in MT_ORDER:
                gi, half = GATE_OF_MT[mt], mt % 2
                if g[gi] is None:
                    g[gi] = gp.tile([128, 2, 512], F32, name=f"g{gi}",
                                    tag=f"g{gi}", bufs=2)
                fn = TANH if gi == 3 else SIG
                nc.scalar.activation(g[gi][:, half, :n], ps(mt, n), fn,
                                     bias=bias[:, mt:mt + 1])
            gv = [t[:, :, :n] for t in g]
            t1 = gp.tile([128, 2, 512], F32, name="t1", tag="t1", bufs=2)
            nc.vector.tensor_mul(t1[:, :, :n], gv[0], gv[3])
            t2 = gp.tile([128, 2, 512], F32, name="t2", tag="t2", bufs=1)
            nc.vector.tensor_mul(t2[:, :, :n], gv[1], ce[:, :, 0, :])
            t3 = gp.tile([128, 2, 512], F32, name="t3", tag="t3", bufs=1)
            nc.vector.tensor_mul(t3[:, :, :n], gv[2], ce[:, :, 1, :])
            s12 = gp.tile([128, 2, 512], F32, name="s12", tag="s12", bufs=1)
            nc.vector.tensor_add(s12[:, :, :n], t1[:, :, :n], t2[:, :, :n])
            nc.vector.tensor_add(couts, s12[:, :, :n], t3[:, :, :n])
            if not defer_h:
                nc.scalar.activation(houts, couts, TANH)

        def compose(hp, cp, houts, couts, n, defer_h=False):
            """One compose sub-chunk (n <= 512 output nodes).
            hp/cp: [128, 2, 2n] views of the previous level."""
            hv = hp.rearrange("p f (n two) -> p f two n", two=2)
            ce = cp.rearrange("p f (n two) -> p f two n", two=2)
            rhs = [hv[:, 0, 0, :], hv[:, 1, 0, :],
                   hv[:, 0, 1, :], hv[:, 1, 1, :]]
            for mt in MT_ORDER:
                for k in range(4):
                    nc.tensor.matmul(ps(mt, n),
                                     wc[k][:, mt * 128:(mt + 1) * 128],
                                     rhs[k], start=(k == 0), stop=(k == 3))
            _gates_and_chain(ce, houts, couts, n, defer_h)

        st = {}  # (level, ch) -> (h_tile, c_tile)

        def emit_leaf(ch):
            h_lf = lvl.tile([128, 2, CHUNK_LEAF], BF16, name="hlf", tag="hlf")
            c_lf = lvl.tile([128, 2, CHUNK_LEAF], F32, name="clf", tag="clf")
            for s in range(CHUNK_LEAF // 512):
                col0 = ch * CHUNK_LEAF + s * 512
                xk01 = xpool.tile([128, 1024], BF16, name="xk01", tag="xk01")
                nc.sync.dma_start(
                    xk01.rearrange("p (k n) -> p k n", k=2),
                    emb_d[0:256, col0:col0 + 512].rearrange(
                        "(k p) n -> p k n", p=128))
                xk2 = xpool.tile([128, 512], BF16, name="xk2", tag="xk2")
                nc.sync.dma_start(xk2[0:45, :],
                                  emb_d[256:301, col0:col0 + 512])
                nc.sync.dma_start(xk2[64:109, :],
                                  emb_d[256:301, col0:col0 + 512])
                po = 2 * (s % 2)      # pairs 0,1 or 2,3
                for mt in range(4):
                    for k in range(2):
                        nc.tensor.matmul(
                            ps(2 * po + mt),
                            wl[:, k * 512 + mt * 128:k * 512 + (mt + 1) * 128],
                            xk01[:, k * 512:(k + 1) * 512],
                            start=(k == 0), stop=False)
                # K=45 tail (44 emb rows + bias row); two m-tiles run
                # concurrently in disjoint PE row groups
                for mt in range(0, 4, 2):
                    nc.tensor.matmul(
                        ps(2 * po + mt), wl2[0:45, mt * 128:(mt + 1) * 128],
                        xk2[0:45, :], start=False, stop=True,
                        tile_position=(0, 0))
                    nc.tensor.matmul(
                        ps(2 * po + mt + 1),
                        wl2[64:109, (mt + 1) * 128:(mt + 2) * 128],
                        xk2[64:109, :], start=False, stop=True,
                        tile_position=(64, 0))
                # drain order: sigmoid first (frees the pair the NEXT
                # leaf sub writes), then the c copy / tanh / h mul
                dst = slice(s * 512, (s + 1) * 512)
                to = gp.tile([128, 1024], BF16, name="lf_to",
                             tag="lf_to", bufs=2)
                nc.scalar.activation(to[:], pairs[po + 1][:], SIG)
                nc.vector.tensor_copy(
                    c_lf[:, :, dst],
                    pairs[po].rearrange("p (f n) -> p f n", f=2))
                tcell = gp.tile([128, 1024], BF16, name="lf_tc",
                                tag="lf_tc", bufs=2)
                nc.scalar.activation(tcell[:], pairs[po][:], TANH)
                nc.vector.tensor_mul(
                    h_lf[:, :, dst],
                    to.rearrange("p (f n) -> p f n", f=2),
                    tcell.rearrange("p (f n) -> p f n", f=2))
            st[(0, ch)] = (h_lf, c_lf)

        def emit_level(li, ch, n_sub=None, h_sub=False):
            """Compose level li (1..3) of chunk ch.  n_sub forces smaller
            sub-chunks and h_sub per-sub h activations (finer pipelining
            for the serial tail of the schedule)."""
            prev_h, prev_c = st.pop((li - 1, ch))
            rows = CHUNK_LEAF >> li
            if li < 3:
                nh = lvl.tile([128, 2, rows], BF16, name=f"h{li}",
                              tag=f"h{li}", bufs=2)
                ncr = lvl.tile([128, 2, rows], F32, name=f"c{li}",
                               tag=f"c{li}", bufs=2)
                off = 0
            else:
                nh, ncr, off = pk_h, pk_c, ch * (CHUNK_LEAF // 8)
            step = n_sub or min(512, rows)
            for j in range(max(1, rows // step)):
                n = min(step, rows - j * step)
                compose(prev_h[:, :, j * 2 * step:j * 2 * step + 2 * n],
                        prev_c[:, :, j * 2 * step:j * 2 * step + 2 * n],
                        nh[:, :, off + j * step:off + j * step + n],
                        ncr[:, :, off + j * step:off + j * step + n],
                        n, defer_h=True)
                if h_sub:
                    nc.scalar.activation(
                        nh[:, :, off + j * step:off + j * step + n],
                        ncr[:, :, off + j * step:off + j * step + n], TANH)
            if not h_sub:
                nc.scalar.activation(nh[:, :, off:off + rows],
                                     ncr[:, :, off:off + rows], TANH)
            if li < 3:
                st[(li, ch)] = (nh, ncr)

        # software-pipelined chunk schedule
        for ch in range(N_CHUNKS):
            emit_leaf(ch)
            if ch == 0:
                for k in range(4):
                    nc.scalar.dma_start(wc[k][:],
                                        wc_d[k * 128:(k + 1) * 128, :])
            if ch >= 1:
                emit_level(2, ch - 1)
            emit_level(1, ch)
            if ch >= 2:
                emit_level(3, ch - 2)
        emit_level(2, N_CHUNKS - 1, n_sub=256, h_sub=True)
        emit_level(3, N_CHUNKS - 2, n_sub=128, h_sub=True)

        # ---- tail levels 4..8 across all trees ----
        def tail_tiles(li, rows):
            nh = lvl.tile([128, 2, rows], BF16, name=f"h{li}", tag=f"h{li}")
            ncr = lvl.tile([128, 2, rows], F32, name=f"c{li}", tag=f"c{li}")
            return nh, ncr

        def tail_sub(prev, cur, j, n):
            prev_h, prev_c = prev
            nh, ncr = cur
            houts = nh[:, :, j * n:j * n + n]
            couts = ncr[:, :, j * n:j * n + n]
            compose(prev_h[:, :, j * 2 * n:j * 2 * n + 2 * n],
                    prev_c[:, :, j * 2 * n:j * 2 * n + 2 * n],
                    houts, couts, n, defer_h=True)
            nc.scalar.activation(houts, couts, TANH)

        l4 = tail_tiles(4, BC * 16)
        l5 = tail_tiles(5, BC * 8)
        l6 = tail_tiles(6, BC * 4)
        l7 = tail_tiles(7, BC * 2)
        tail_sub((pk_h, pk_c), l4, 0, 512)
        emit_level(3, N_CHUNKS - 1, n_sub=128, h_sub=True)
        tail_sub((pk_h, pk_c), l4, 2, 256)
        tail_sub((pk_h, pk_c), l4, 3, 256)
        tail_sub(l4, l5, 0, 256)
        tail_sub(l4, l5, 1, 256)
        tail_sub(l5, l6, 0, 128)
        tail_sub(l5, l6, 1, 128)
        tail_sub(l6, l7, 0, 64)
        tail_sub(l6, l7, 1, 64)
        stage_c = lvl.tile([128, 2, BC], F32, name="stc", tag="stc")
        stage_h = lvl.tile([128, 2, BC], F32, name="sth", tag="sth")
        for j in range(2):
            compose(l7[0][:, :, j * 64:j * 64 + 64],
                    l7[1][:, :, j * 64:j * 64 + 64],
                    stage_h[:, :, j * 32:j * 32 + 32],
                    stage_c[:, :, j * 32:j * 32 + 32], 32)
        for p in range(2):
            nc.sync.dma_start(outc_d[p * 128:(p + 1) * 128, :],
                              stage_c[:, p, :])
            nc.sync.dma_start(outh_d[p * 128:(p + 1) * 128, :],
                              stage_h[:, p, :])

    nc.compile()
    return nc


def _prep_inputs(embs, cx_w, cx_b, ox_w, ox_b, lh_w, lh_b, rh_w, rh_b):
    bf16 = ml_dtypes.bfloat16
    w_leaf = np.ascontiguousarray(
        np.concatenate([cx_w, ox_w], axis=0).T).astype(np.float32)  # [300,512]
    leaf_b = np.concatenate([cx_b, ox_b]).astype(np.float32)        # [512]
    # main k-tiles side by side: [128, 1024] = [k0 | k1]
    wl = np.concatenate([w_leaf[0:128], w_leaf[128:256]],
                        axis=1).astype(bf16)
    # K=45 tail (rows 256:300 + bias row), duplicated at partitions 64..108
    wl2 = np.zeros((128, 2 * M), np.float32)
    wl2[0:44] = w_leaf[256:300]
    wl2[44] = leaf_b
    wl2[64:108] = w_leaf[256:300]
    wl2[108] = leaf_b
    wl2 = wl2.astype(bf16)
    w_comp = np.ascontiguousarray(np.concatenate(
        [lh_w.reshape(4 * M, M).T, rh_w.reshape(4 * M, M).T],
        axis=0)).astype(bf16)                                       # [512,1024]
    b_comp = (lh_b + rh_b).reshape(-1).astype(np.float32)           # [1024]
    biases = np.ascontiguousarray(b_comp.reshape(8, 128).T)         # [128, 8]
    embs_sh = np.asarray(embs, np.float32).reshape(N_CORES, LEAF_ROWS, IN)
    in_maps = []
    for c in range(N_CORES):
        xt = np.empty((IN + 1, LEAF_ROWS), np.float32)
        xt[0:IN] = embs_sh[c].T
        xt[IN] = 1.0
        in_maps.append({"embs_t": np.ascontiguousarray(xt.astype(bf16)),
                        "w_leaf": wl, "w_leaf2": wl2,
                        "w_comp": w_comp, "biases": biases})
    return in_maps


def kernel(embs, cx_w, cx_b, ox_w, ox_b, lh_w, lh_b, rh_w, rh_b):
    global LAST_RESULTS
    from concourse.bass_utils import run_bass_kernel_spmd

    if "nc" not in _CACHE:
        _CACHE["nc"] = _build()
    nc = _CACHE["nc"]
    in_maps = _prep_inputs(embs, cx_w, cx_b, ox_w, ox_b,
                           lh_w, lh_b, rh_w, rh_b)
    res = run_bass_kernel_spmd(nc, in_maps, core_ids=list(range(N_CORES)))
    LAST_RESULTS = res
    c_out = np.empty((B, M), np.float32)
    h_out = np.empty((B, M), np.float32)
    for c in range(N_CORES):
        c_out[c * BC:(c + 1) * BC] = res.results[c]["out_c"].T
        h_out[c * BC:(c + 1) * BC] = res.results[c]["out_h"].T
    return c_out, h_out


# revision 11
# speedup vs baseline: 1.0486x; 1.0204x over previous
"""BinaryTreeLSTM Trainium2 kernel — data-parallel over the batch (tree) axis.

Layout strategy: activations in [feature, row] layout on-chip; TensorE
matmul PSUM output [out_feat, row] is exactly the next level's moving
operand layout, so no on-device transposes.  The host transposes embs
once and packs/casts weights.

v2 structure (vs v1 baseline):
 - Leaf biases are folded into the K=45 tail matmul (host appends a
   ones-row to embs and a bias-row to the tail weights), so the leaf
   needs no per-element bias add and the two [128,512] PSUM banks of
   each gate can be activated with ONE [128,1024] instruction.
 - PSUM is organized as four 2-bank [128,1024] pair tiles.
 - Per level, both feature halves live in ONE tile ([128, 2, rows]),
   so each DVE c-chain op covers both halves in one instruction
   (amortizes the ~150-cycle per-op bubble).
 - Gates are stored bf16 (enables 16-bit DVE/ACT fast paths; rel-err
   budget checked by simulation: ~1.4e-2 vs the 2e-2 gate).
 - PE warm-up: gpsimd memset (earliest engine ready) + 10 dummy
   matmuls so HAM un-throttles before real work lands.
 - Tail levels 5..8 run as single wide compose calls.

dtypes: matmul operands bf16 (fp32 PSUM accumulate); cell state c and
the DVE chain stay fp32 end-to-end; gates/h stored bf16.
"""

import sys

if "/opt/trn_rl_repo" not in sys.path:
    sys.path.insert(0, "/opt/trn_rl_repo")

from contextlib import ExitStack

import ml_dtypes
import numpy as np

N_CORES = 8
B, L, IN, M = 512, 256, 300, 256
BC = B // N_CORES            # trees per core
LEAF_ROWS = BC * L           # 16384 leaf rows per core
T_CHUNK = 8                  # trees per chunk
N_CHUNKS = BC // T_CHUNK
CHUNK_LEAF = T_CHUNK * L     # 2048 leaf rows per chunk
N_DUMMY = 6                  # HAM warm-up matmuls

_CACHE = {}
LAST_RESULTS = None


def _build():
    import concourse.bass as bass  # noqa: F401
    import concourse.tile as tile
    from concourse import bacc, mybir

    F32 = mybir.dt.float32
    BF16 = mybir.dt.bfloat16
    SIG = mybir.ActivationFunctionType.Sigmoid
    TANH = mybir.ActivationFunctionType.Tanh

    nc = bacc.Bacc("TRN2", target_bir_lowering=False, debug=False,
                   num_devices=N_CORES)
    emb_d = nc.dram_tensor("embs_t", [IN + 1, LEAF_ROWS], BF16,
                           kind="ExternalInput").ap()
    wl_d = nc.dram_tensor("w_leaf", [128, 1024], BF16,
                          kind="ExternalInput").ap()
    wl2_d = nc.dram_tensor("w_leaf2", [128, 2 * M], BF16,
                           kind="ExternalInput").ap()
    wc_d = nc.dram_tensor("w_comp", [2 * M, 4 * M], BF16,
                          kind="ExternalInput").ap()
    b_d = nc.dram_tensor("biases", [128, 8], F32, kind="ExternalInput").ap()
    outc_d = nc.dram_tensor("out_c", [M, BC], F32, kind="ExternalOutput").ap()
    outh_d = nc.dram_tensor("out_h", [M, BC], F32, kind="ExternalOutput").ap()

    with tile.TileContext(nc) as tc, ExitStack() as ctx:
        wpool = ctx.enter_context(tc.tile_pool(name="w", bufs=1))
        xpool = ctx.enter_context(tc.tile_pool(name="x", bufs=5))
        lvl = ctx.enter_context(tc.tile_pool(name="lvl", bufs=1))
        gp = ctx.enter_context(tc.tile_pool(name="g", bufs=2))
        pp = ctx.enter_context(tc.tile_pool(name="ps", bufs=1, space="PSUM"))

        # --- PE warm-up: memset on gpsimd (earliest engine out of its
        # preamble) so dummy matmuls can start ~6us in, covering the
        # initial DMA wait and warming the HAM clock gate ---
        dummy = wpool.tile([128, 512], BF16, name="dummy", tag="dummy")
        nc.gpsimd.memset(dummy[:], 0.0)

        # --- weights / biases (resident) on the ACT HWDGE queue; the SP
        # queue leads with the first leaf's embs tiles ---
        wl = wpool.tile([128, 1024], BF16, name="wl", tag="wl")
        nc.scalar.dma_start(wl[:], wl_d[:, :])
        wl2 = wpool.tile([128, 2 * M], BF16, name="wl2", tag="wl2")
        nc.scalar.dma_start(wl2[:], wl2_d[:, :])
        bias = wpool.tile([128, 8], F32, name="bias", tag="bias")
        nc.scalar.dma_start(bias[:], b_d[:, :])
        wc = [wpool.tile([128, 4 * M], BF16, name=f"wc{k}", tag=f"wc{k}")
              for k in range(4)]

        # PSUM: four [128, 1024] 2-bank pair tiles
        pairs = [pp.tile([128, 1024], F32, name=f"pp{i}", tag=f"pp{i}")
                 for i in range(4)]

        def ps(mt, n=512):
            return pairs[mt // 2][:, (mt % 2) * 512:(mt % 2) * 512 + n]

        # dummies on pairs 2,3 (banks 4-7); first leaf sub uses pairs 0,1
        for i in range(N_DUMMY):
            nc.tensor.matmul(ps(4 + (i % 4)), dummy[:, 0:128],
                             dummy[:], start=True, stop=True)

        # --- parked L3 state (all trees) ---
        pk_h = lvl.tile([128, 2, BC * 32], BF16, name="pkh", tag="pkh")
        pk_c = lvl.tile([128, 2, BC * 32], F32, name="pkc", tag="pkc")

        # gate m-tile order: wc columns are [i0,i1,lf0,lf1,rf0,rf1,u0,u1];
        # i and u first so the DVE t1 = i*u can start earliest.
        MT_ORDER = (0, 1, 6, 7, 2, 3, 4, 5)
        GATE_OF_MT = (0, 0, 1, 1, 2, 2, 3, 3)   # i, lf, rf, u

        # deferred h = tanh(c) pieces: emitted into the ACT queue right
        # after the NEXT compose's gate batch, where the queue has slack
        # and the DVE chain that produces c has long finished.
        pending_h = []

        def flush_pending():
            for ho, co in pending_h:
                nc.scalar.activation(ho, co, TANH)
            pending_h.clear()

        def _gates_and_chain(ce, houts, couts, n, h_mode):
            """Gate activations + c-chain for one <=512-node sub-chunk.
            ce: [128, 2, 2, n] view of prev c (dim2 = even/odd);
            houts/couts: [128, 2, n] views."""
            g = [None] * 4   # i, lf, rf, u tiles [128, 2, 512] fp32
            for mt in MT_ORDER:
                gi, half = GATE_OF_MT[mt], mt % 2
                if g[gi] is None:
                    g[gi] = gp.tile([128, 2, 512], F32, name=f"g{gi}",
                                    tag=f"g{gi}", bufs=2)
                fn = TANH if gi == 3 else SIG
                nc.scalar.activation(g[gi][:, half, :n], ps(mt, n), fn,
                                     bias=bias[:, mt:mt + 1])
            flush_pending()
            gv = [t[:, :, :n] for t in g]
            t1 = gp.tile([128, 2, 512], F32, name="t1", tag="t1", bufs=2)
            nc.vector.tensor_mul(t1[:, :, :n], gv[0], gv[3])
            t2 = gp.tile([128, 2, 512], F32, name="t2", tag="t2", bufs=1)
            nc.vector.tensor_mul(t2[:, :, :n], gv[1], ce[:, :, 0, :])
            t3 = gp.tile([128, 2, 512], F32, name="t3", tag="t3", bufs=1)
            nc.vector.tensor_mul(t3[:, :, :n], gv[2], ce[:, :, 1, :])
            s12 = gp.tile([128, 2, 512], F32, name="s12", tag="s12", bufs=1)
            nc.vector.tensor_add(s12[:, :, :n], t1[:, :, :n], t2[:, :, :n])
            nc.vector.tensor_add(couts, s12[:, :, :n], t3[:, :, :n])
            if h_mode == "pending":
                pending_h.append((houts, couts))
            else:   # immediate, per half for lowest next-consumer latency
                nc.scalar.activation(houts[:, 0, :], couts[:, 0, :], TANH)
                nc.scalar.activation(houts[:, 1, :], couts[:, 1, :], TANH)

        def compose(hp, cp, houts, couts, n, h_mode="immediate"):
            """One compose sub-chunk (n <= 512 output nodes).
            hp/cp: [128, 2, 2n] views of the previous level."""
            hv = hp.rearrange("p f (n two) -> p f two n", two=2)
            ce = cp.rearrange("p f (n two) -> p f two n", two=2)
            rhs = [hv[:, 0, 0, :], hv[:, 1, 0, :],
                   hv[:, 0, 1, :], hv[:, 1, 1, :]]
            for mt in MT_ORDER:
                for k in range(4):
                    nc.tensor.matmul(ps(mt, n),
                                     wc[k][:, mt * 128:(mt + 1) * 128],
                                     rhs[k], start=(k == 0), stop=(k == 3))
            _gates_and_chain(ce, houts, couts, n, h_mode)

        st = {}  # (level, ch) -> (h_tile, c_tile)

        def emit_leaf(ch):
            h_lf = lvl.tile([128, 2, CHUNK_LEAF], BF16, name="hlf", tag="hlf")
            c_lf = lvl.tile([128, 2, CHUNK_LEAF], F32, name="clf", tag="clf")
            for s in range(CHUNK_LEAF // 512):
                col0 = ch * CHUNK_LEAF + s * 512
                xk01 = xpool.tile([128, 1024], BF16, name="xk01", tag="xk01")
                nc.sync.dma_start(
                    xk01.rearrange("p (k n) -> p k n", k=2),
                    emb_d[0:256, col0:col0 + 512].rearrange(
                        "(k p) n -> p k n", p=128))
                xk2 = xpool.tile([128, 512], BF16, name="xk2", tag="xk2")
                nc.sync.dma_start(xk2[0:45, :],
                                  emb_d[256:301, col0:col0 + 512])
                nc.sync.dma_start(xk2[64:109, :],
                                  emb_d[256:301, col0:col0 + 512])
                po = 2 * (s % 2)      # pairs 0,1 or 2,3
                for mt in range(4):
                    for k in range(2):
                        nc.tensor.matmul(
                            ps(2 * po + mt),
                            wl[:, k * 512 + mt * 128:k * 512 + (mt + 1) * 128],
                            xk01[:, k * 512:(k + 1) * 512],
                            start=(k == 0), stop=False)
                # K=45 tail (44 emb rows + bias row); two m-tiles run
                # concurrently in disjoint PE row groups
                for mt in range(0, 4, 2):
                    nc.tensor.matmul(
                        ps(2 * po + mt), wl2[0:45, mt * 128:(mt + 1) * 128],
                        xk2[0:45, :], start=False, stop=True,
                        tile_position=(0, 0))
                    nc.tensor.matmul(
                        ps(2 * po + mt + 1),
                        wl2[64:109, (mt + 1) * 128:(mt + 2) * 128],
                        xk2[64:109, :], start=False, stop=True,
                        tile_position=(64, 0))
                # drain order: sigmoid first (frees the pair the NEXT
                # leaf sub writes), then the c copy / tanh / h mul
                dst = slice(s * 512, (s + 1) * 512)
                to = gp.tile([128, 1024], BF16, name="lf_to",
                             tag="lf_to", bufs=2)
                nc.scalar.activation(to[:], pairs[po + 1][:], SIG)
                nc.vector.tensor_copy(
                    c_lf[:, :, dst],
                    pairs[po].rearrange("p (f n) -> p f n", f=2))
                tcell = gp.tile([128, 1024], BF16, name="lf_tc",
                                tag="lf_tc", bufs=2)
                nc.scalar.activation(tcell[:], pairs[po][:], TANH)
                nc.vector.tensor_mul(
                    h_lf[:, :, dst],
                    to.rearrange("p (f n) -> p f n", f=2),
                    tcell.rearrange("p (f n) -> p f n", f=2))
            st[(0, ch)] = (h_lf, c_lf)

        def emit_level(li, ch, h_sub=False):
            """Compose level li (1..3) of chunk ch.  h_sub=True emits
            h immediately per sub (for the serial tail)."""
            if li == 2 or h_sub:
                flush_pending()
            prev_h, prev_c = st.pop((li - 1, ch))
            rows = CHUNK_LEAF >> li
            if li < 3:
                nh = lvl.tile([128, 2, rows], BF16, name=f"h{li}",
                              tag=f"h{li}", bufs=2)
                ncr = lvl.tile([128, 2, rows], F32, name=f"c{li}",
                               tag=f"c{li}", bufs=2)
                off = 0
            else:
                nh, ncr, off = pk_h, pk_c, ch * (CHUNK_LEAF // 8)
            step = min(512, rows)
            for j in range(max(1, rows // step)):
                n = min(step, rows - j * step)
                compose(prev_h[:, :, j * 2 * step:j * 2 * step + 2 * n],
                        prev_c[:, :, j * 2 * step:j * 2 * step + 2 * n],
                        nh[:, :, off + j * step:off + j * step + n],
                        ncr[:, :, off + j * step:off + j * step + n],
                        n, h_mode="immediate" if h_sub else "pending")
            if li < 3:
                st[(li, ch)] = (nh, ncr)

        # ---- tail level tiles (levels 4..8 across all trees) ----
        def tail_tiles(li, rows):
            nh = lvl.tile([128, 2, rows], BF16, name=f"h{li}", tag=f"h{li}")
            ncr = lvl.tile([128, 2, rows], F32, name=f"c{li}", tag=f"c{li}")
            return nh, ncr

        def tail_sub(prev, cur, j, n):
            flush_pending()
            prev_h, prev_c = prev
            nh, ncr = cur
            compose(prev_h[:, :, j * 2 * n:j * 2 * n + 2 * n],
                    prev_c[:, :, j * 2 * n:j * 2 * n + 2 * n],
                    nh[:, :, j * n:j * n + n],
                    ncr[:, :, j * n:j * n + n], n, h_mode="immediate")

        l4 = tail_tiles(4, BC * 16)
        l5 = tail_tiles(5, BC * 8)
        l6 = tail_tiles(6, BC * 4)
        l7 = tail_tiles(7, BC * 2)

        # software-pipelined chunk schedule; the first l4/l5 tail subs are
        # interleaved into late iterations so the serial tail shrinks
        for ch in range(N_CHUNKS):
            emit_leaf(ch)
            if ch == 0:
                for k in range(4):
                    nc.scalar.dma_start(wc[k][:],
                                        wc_d[k * 128:(k + 1) * 128, :])
            if ch >= 1:
                emit_level(2, ch - 1)
            if ch == 6:
                tail_sub((pk_h, pk_c), l4, 0, 512)   # needs L3(0..3)
            if ch == 7:
                tail_sub(l4, l5, 0, 256)             # needs l4 j0
            emit_level(1, ch)
            if ch >= 2:
                emit_level(3, ch - 2)
        emit_level(2, N_CHUNKS - 1, h_sub=True)
        emit_level(3, N_CHUNKS - 2, h_sub=True)
        emit_level(3, N_CHUNKS - 1, h_sub=True)
        tail_sub((pk_h, pk_c), l4, 2, 256)
        tail_sub((pk_h, pk_c), l4, 3, 256)
        tail_sub(l4, l5, 1, 256)
        tail_sub(l5, l6, 0, 128)
        tail_sub(l5, l6, 1, 128)
        tail_sub(l6, l7, 0, 64)
        tail_sub(l6, l7, 1, 64)
        stage_c = lvl.tile([128, 2, BC], F32, name="stc", tag="stc")
        stage_h = lvl.tile([128, 2, BC], F32, name="sth", tag="sth")
        for j in range(2):
            compose(l7[0][:, :, j * 64:j * 64 + 64],
                    l7[1][:, :, j * 64:j * 64 + 64],
                    stage_h[:, :, j * 32:j * 32 + 32],
                    stage_c[:, :, j * 32:j * 32 + 32], 32)
        for p in range(2):
            nc.sync.dma_start(outc_d[p * 128:(p + 1) * 128, :],
                              stage_c[:, p, :])
            nc.sync.dma_start(outh_d[p * 128:(p + 1) * 128, :],
                              stage_h[:, p, :])

    nc.compile()
    return nc


def _prep_inputs(embs, cx_w, cx_b, ox_w, ox_b, lh_w, lh_b, rh_w, rh_b):
    bf16 = ml_dtypes.bfloat16
    w_leaf = np.ascontiguousarray(
        np.concatenate([cx_w, ox_w], axis=0).T).astype(np.float32)  # [300,512]
    leaf_b = np.concatenate([cx_b, ox_b]).astype(np.float32)        # [512]
    # main k-tiles side by side: [128, 1024] = [k0 | k1]
    wl = np.concatenate([w_leaf[0:128], w_leaf[128:256]],
                        axis=1).astype(bf16)
    # K=45 tail (rows 256:300 + bias row), duplicated at partitions 64..108
    wl2 = np.zeros((128, 2 * M), np.float32)
    wl2[0:44] = w_leaf[256:300]
    wl2[44] = leaf_b
    wl2[64:108] = w_leaf[256:300]
    wl2[108] = leaf_b
    wl2 = wl2.astype(bf16)
    w_comp = np.ascontiguousarray(np.concatenate(
        [lh_w.reshape(4 * M, M).T, rh_w.reshape(4 * M, M).T],
        axis=0)).astype(bf16)                                       # [512,1024]
    b_comp = (lh_b + rh_b).reshape(-1).astype(np.float32)           # [1024]
    biases = np.ascontiguousarray(b_comp.reshape(8, 128).T)         # [128, 8]
    embs_sh = np.asarray(embs, np.float32).reshape(N_CORES, LEAF_ROWS, IN)
    in_maps = []
    for c in range(N_CORES):
        xt = np.empty((IN + 1, LEAF_ROWS), np.float32)
        xt[0:IN] = embs_sh[c].T
        xt[IN] = 1.0
        in_maps.append({"embs_t": np.ascontiguousarray(xt.astype(bf16)),
                        "w_leaf": wl, "w_leaf2": wl2,
                        "w_comp": w_comp, "biases": biases})
    return in_maps


def kernel(embs, cx_w, cx_b, ox_w, ox_b, lh_w, lh_b, rh_w, rh_b):
    global LAST_RESULTS
    from concourse.bass_utils import run_bass_kernel_spmd

    if "nc" not in _CACHE:
        _CACHE["nc"] = _build()
    nc = _CACHE["nc"]
    in_maps = _prep_inputs(embs, cx_w, cx_b, ox_w, ox_b,
                           lh_w, lh_b, rh_w, rh_b)
    res = run_bass_kernel_spmd(nc, in_maps, core_ids=list(range(N_CORES)))
    LAST_RESULTS = res
    c_out = np.empty((B, M), np.float32)
    h_out = np.empty((B, M), np.float32)
    for c in range(N_CORES):
        c_out[c * BC:(c + 1) * BC] = res.results[c]["out_c"].T
        h_out[c * BC:(c + 1) * BC] = res.results[c]["out_h"].T
    return c_out, h_out
